# revision 10
# baseline (speedup 1.0000x reference)
"""Trainium2 8-core attention kernel (N=8192, D=512, Q==K shared projection).

Strategy: sequence-parallel attention. Each core receives the FULL transposed
embedding (replicated) plus its local 1024-column slice, computes full
K^T = (E @ W_qk^T + b_qk)^T and V = E @ W_v^T locally in bf16 (projections are
~6% of total FLOPs so replicating them beats a collective), then computes its
own [1024, 8192] score tile in the TRANSPOSED layout S^T[c, r].  In that
layout the exp'd scores are directly the lhsT of the P@V matmul (no on-device
transposes at all), and the softmax denominator comes from a ones-vector
matmul.  Softmax max-subtraction is skipped: scores are bounded (~13) so exp
stays well inside fp32/bf16 range; V's bias is folded in after normalization
(P@(V+b) = P@V + l*b  =>  out = P@V/l + b_v).
"""

import numpy as np

import concourse.bass as bass
import concourse.mybir as mybir
import concourse.tile as tile
from concourse import bacc
from concourse.bass_utils import run_bass_kernel_spmd

N = 8192          # sequence length
F = 512           # input features
D = 512           # output features (head dim)
CORES = 8
NL = N // CORES   # local rows per core (1024)
SCALE = 1.0 / float(np.sqrt(D))

FC = F // 128     # 4 f-chunks (contraction for projections)
DC = D // 128     # 4 d-chunks (partition chunks of K^T)
NOCT = 8          # process n in 8 octants during projections
OW = N // NOCT    # 1024 octant width
RB = NL // 512    # 2 row-blocks of 512
CC = N // 128     # 64 column chunks in attention

f32 = mybir.dt.float32
bf16 = mybir.dt.bfloat16

_NC = None
LAST_RESULT = None


def build_kernel():
    nc = bacc.Bacc(target_bir_lowering=False)

    embT = nc.declare_dram_parameter("embT", [F, N], f32, isOutput=False)
    embTl = nc.declare_dram_parameter("embTl", [F, NL], f32, isOutput=False)
    wqkT = nc.declare_dram_parameter("wqkT", [F, D], f32, isOutput=False)
    wvT = nc.declare_dram_parameter("wvT", [F, D], f32, isOutput=False)
    bqk = nc.declare_dram_parameter("bqk", [D], f32, isOutput=False)
    bv = nc.declare_dram_parameter("bv", [D], f32, isOutput=False)
    out = nc.declare_dram_parameter("out", [NL, D], f32, isOutput=True)

    with tile.TileContext(nc) as tc:
        with (
            tc.tile_pool(name="persist", bufs=1) as persist,
            tc.tile_pool(name="stage", bufs=2) as stage,
            tc.tile_pool(name="work", bufs=2) as work,
            tc.tile_pool(name="ps", bufs=2, space="PSUM") as ps,
        ):
            # ---- constants / weights ----
            # w*T chunks: f-chunk fc lives at cols [fc*D, (fc+1)*D)
            wq = persist.tile([128, FC * D], bf16)
            wv = persist.tile([128, FC * D], bf16)
            for src, dst in ((wqkT, wq), (wvT, wv)):
                for fc in range(FC):
                    w_f32 = stage.tile([128, D], f32, tag="wstage")
                    nc.gpsimd.dma_start(
                        out=w_f32, in_=src[fc * 128:(fc + 1) * 128, :])
                    nc.vector.tensor_copy(
                        out=dst[:, fc * D:(fc + 1) * D], in_=w_f32)

            # b_qk as [p, dchunk]: bqk_t[p, dc] = b_qk[dc*128 + p]
            bqk_t = persist.tile([128, DC], f32)
            nc.gpsimd.dma_start(out=bqk_t, in_=bqk.rearrange("(c p) -> p c", p=128))

            # b_v broadcast across partitions: [128, D]
            bv_bc = persist.tile([128, D], f32)
            bv_ap = bv[:]
            bv_bcast = bass.AP(
                tensor=bv_ap.tensor, offset=bv_ap.offset,
                ap=[[0, 128], *bv_ap.ap],
            )
            nc.gpsimd.dma_start(out=bv_bc, in_=bv_bcast)

            ones = persist.tile([128, 1], bf16)
            nc.vector.memset(ones, 1.0)
            zeros4 = persist.tile([128, 4], bf16)
            nc.vector.memset(zeros4, 0.0)

            # ---- persistent big tensors ----
            # K^T: d-chunk dc at cols [dc*N, (dc+1)*N)   (bf16, 64KB/partition)
            kt = persist.tile([128, DC * N], bf16)
            # V: c-chunk cc at cols [cc*D, (cc+1)*D)     (bf16, 64KB/partition)
            v = persist.tile([128, CC * D], bf16)
            # local Q^T: d-chunk dc at cols [dc*NL, (dc+1)*NL)
            qtl = persist.tile([128, DC * NL], bf16)

            # ---- phase 1a: local Q^T (small, warms up PE) ----
            etl_f32 = stage.tile([128, NL], f32, tag="etstage")
            etl = work.tile([128, FC * NL], bf16, tag="etl", bufs=1)
            for fc in range(FC):
                nc.gpsimd.dma_start(
                    out=etl_f32, in_=embTl[fc * 128:(fc + 1) * 128, :])
                nc.vector.tensor_copy(
                    out=etl[:, fc * NL:(fc + 1) * NL], in_=etl_f32)
                etl_f32 = stage.tile([128, NL], f32, tag="etstage")
            for dc in range(DC):
                for nb in range(NL // 512):
                    qtl_ps = ps.tile([128, 512], f32, tag="mm_ps")
                    for fc in range(FC):
                        nc.tensor.matmul(
                            qtl_ps,
                            wq[:, fc * D + dc * 128: fc * D + (dc + 1) * 128],
                            etl[:, fc * NL + nb * 512: fc * NL + (nb + 1) * 512],
                            start=(fc == 0), stop=(fc == FC - 1),
                        )
                    nc.scalar.activation(
                        out=qtl[:, dc * NL + nb * 512: dc * NL + (nb + 1) * 512],
                        in_=qtl_ps,
                        func=mybir.ActivationFunctionType.Identity,
                        bias=bqk_t[:, dc:dc + 1],
                    )

            # ---- phase 1b: full K^T and V, in n-octants ----
            for oct_ in range(NOCT):
                n0 = oct_ * OW
                # load + cast one octant of embT: f-chunk fc at cols [fc*OW, ...)
                et = work.tile([128, FC * OW], bf16, tag="et")
                for fc in range(FC):
                    et_f32 = stage.tile([128, OW], f32, tag="etstage")
                    nc.gpsimd.dma_start(
                        out=et_f32,
                        in_=embT[fc * 128:(fc + 1) * 128, n0:n0 + OW],
                    )
                    nc.vector.tensor_copy(
                        out=et[:, fc * OW:(fc + 1) * OW], in_=et_f32)

                # K^T octant: for each d-chunk, 2 n-blocks of 512
                for dc in range(DC):
                    for nb in range(OW // 512):
                        kt_ps = ps.tile([128, 512], f32, tag="mm_ps")
                        for fc in range(FC):
                            nc.tensor.matmul(
                                kt_ps,
                                wq[:, fc * D + dc * 128: fc * D + (dc + 1) * 128],
                                et[:, fc * OW + nb * 512: fc * OW + (nb + 1) * 512],
                                start=(fc == 0), stop=(fc == FC - 1),
                            )
                        col = dc * N + n0 + nb * 512
                        nc.scalar.activation(
                            out=kt[:, col:col + 512],
                            in_=kt_ps,
                            func=mybir.ActivationFunctionType.Identity,
                            bias=bqk_t[:, dc:dc + 1],
                        )

                # V octant: 8 c-chunks of 128
                for ci in range(OW // 128):
                    v_ps = ps.tile([128, 512], f32, tag="mm_ps")
                    for fc in range(FC):
                        nc.tensor.matmul(
                            v_ps,
                            et[:, fc * OW + ci * 128: fc * OW + (ci + 1) * 128],
                            wv[:, fc * D:(fc + 1) * D],
                            start=(fc == 0), stop=(fc == FC - 1),
                        )
                    cc = oct_ * (OW // 128) + ci
                    nc.scalar.activation(
                        out=v[:, cc * D:(cc + 1) * D],
                        in_=v_ps,
                        func=mybir.ActivationFunctionType.Copy,
                    )

            # ---- phase 2: attention, 2 row-blocks of 512 ----
            for rb in range(RB):
                r0 = rb * 512
                pv_ps = [
                    ps.tile([128, D], f32, tag="pv_ps", bufs=4, name=f"pv{rb}_{j}")
                    for j in range(4)
                ]
                l_ps = ps.tile([128, 4], f32, tag="l_ps", bufs=1)
                # One accumulation group per PSUM bank: start=True clears the
                # whole bank's has_written bits, so interleaved per-column
                # groups lose their first-chunk term.  Prime the bank with a
                # single zero matmul, then accumulate everything start=False.
                nc.tensor.matmul(
                    l_ps, wq[:, 0:128], zeros4,
                    start=True, stop=False, skip_group_check=True,
                )
                for cc in range(CC):
                    st_ps = ps.tile([128, 512], f32, tag="mm_ps")
                    for dc in range(DC):
                        nc.tensor.matmul(
                            st_ps,
                            kt[:, dc * N + cc * 128: dc * N + (cc + 1) * 128],
                            qtl[:, dc * NL + r0: dc * NL + r0 + 512],
                            start=(dc == 0), stop=(dc == DC - 1),
                        )
                    p_t = work.tile([128, 512], bf16, tag="p_t", bufs=3)
                    nc.scalar.activation(
                        out=p_t, in_=st_ps,
                        func=mybir.ActivationFunctionType.Exp,
                        scale=SCALE,
                    )
                    for j in range(4):
                        nc.tensor.matmul(
                            pv_ps[j],
                            p_t[:, j * 128:(j + 1) * 128],
                            v[:, cc * D:(cc + 1) * D],
                            start=(cc == 0), stop=(cc == CC - 1),
                        )
                    for j in range(4):
                        nc.tensor.matmul(
                            l_ps[:, j:j + 1],
                            p_t[:, j * 128:(j + 1) * 128],
                            ones,
                            start=False, stop=(cc == CC - 1),
                            skip_group_check=True,
                        )

                linv = work.tile([128, 4], f32, tag="linv")
                nc.vector.reciprocal(out=linv, in_=l_ps)
                for j in range(4):
                    o_t = work.tile([128, D], f32, tag="o_t", bufs=3)
                    nc.scalar.activation(
                        out=o_t, in_=pv_ps[j],
                        func=mybir.ActivationFunctionType.Copy,
                        scale=linv[:, j:j + 1],
                    )
                    nc.vector.tensor_add(o_t, o_t, bv_bc)
                    nc.gpsimd.dma_start(
                        out=out[r0 + j * 128: r0 + (j + 1) * 128, :], in_=o_t)

    nc.compile()
    return nc


def _get_nc():
    global _NC
    if _NC is None:
        _NC = build_kernel()
    return _NC


def kernel(embedding, W_qk, b_qk, W_v, b_v):
    global LAST_RESULT
    embedding = np.ascontiguousarray(np.asarray(embedding, dtype=np.float32))
    embT = np.ascontiguousarray(embedding.T)
    wqkT = np.ascontiguousarray(np.asarray(W_qk, dtype=np.float32).T)
    wvT = np.ascontiguousarray(np.asarray(W_v, dtype=np.float32).T)
    bqk = np.ascontiguousarray(np.asarray(b_qk, dtype=np.float32))
    bv = np.ascontiguousarray(np.asarray(b_v, dtype=np.float32))

    in_maps = [
        {
            "embT": embT,
            "embTl": np.ascontiguousarray(embT[:, i * NL:(i + 1) * NL]),
            "wqkT": wqkT,
            "wvT": wvT,
            "bqk": bqk,
            "bv": bv,
        }
        for i in range(CORES)
    ]

    nc = _get_nc()
    res = run_bass_kernel_spmd(nc, in_maps, core_ids=list(range(CORES)))
    LAST_RESULT = res
    return np.concatenate(
        [np.asarray(res.results[i]["out"]) for i in range(CORES)], axis=0
    )


# revision 12
# speedup vs baseline: 1.0621x; 1.0621x over previous
"""Trainium2 8-core attention kernel (N=8192, D=512, Q==K shared projection).

Strategy: sequence-parallel attention. Each core receives the FULL transposed
embedding (replicated) plus its local 1024-column slice, computes full
K^T = (E @ W_qk^T + b_qk)^T and V = E @ W_v^T locally in bf16 (projections are
~6% of total FLOPs so replicating them beats a collective), then computes its
own [1024, 8192] score tile in the TRANSPOSED layout S^T[c, r].  In that
layout the exp'd scores are directly the lhsT of the P@V matmul (no on-device
transposes at all), and the softmax denominator comes from a ones-vector
matmul.  Softmax max-subtraction is skipped: scores are bounded (~13) so exp
stays well inside fp32/bf16 range; V's bias is folded in after normalization
(P@(V+b) = P@V + l*b  =>  out = P@V/l + b_v).
"""

import ml_dtypes
import numpy as np

import concourse.bass as bass
import concourse.mybir as mybir
import concourse.tile as tile
from concourse import bacc
from concourse.bass_utils import run_bass_kernel_spmd

N = 8192          # sequence length
F = 512           # input features
D = 512           # output features (head dim)
CORES = 8
NL = N // CORES   # local rows per core (1024)
SCALE = 1.0 / float(np.sqrt(D))

FC = F // 128     # 4 f-chunks (contraction for projections)
DC = D // 128     # 4 d-chunks (partition chunks of K^T)
NOCT = 8          # process n in 8 octants during projections
OW = N // NOCT    # 1024 octant width
RB = NL // 512    # 2 row-blocks of 512
CC = N // 128     # 64 column chunks in attention

f32 = mybir.dt.float32
bf16 = mybir.dt.bfloat16

_NC = None
LAST_RESULT = None


def build_kernel():
    nc = bacc.Bacc(target_bir_lowering=False)

    embT = nc.declare_dram_parameter("embT", [F, N], bf16, isOutput=False)
    embTl = nc.declare_dram_parameter("embTl", [F, NL], bf16, isOutput=False)
    wqkT = nc.declare_dram_parameter("wqkT", [F, D], bf16, isOutput=False)
    wvT = nc.declare_dram_parameter("wvT", [F, D], bf16, isOutput=False)
    bqk = nc.declare_dram_parameter("bqk", [D], f32, isOutput=False)
    bv = nc.declare_dram_parameter("bv", [D], f32, isOutput=False)
    out = nc.declare_dram_parameter("out", [NL, D], f32, isOutput=True)

    with tile.TileContext(nc) as tc:
        with (
            tc.tile_pool(name="persist", bufs=1) as persist,
            tc.tile_pool(name="work", bufs=2) as work,
            tc.tile_pool(name="ps", bufs=2, space="PSUM") as ps,
        ):
            # ---- constants / weights ----
            # w*T chunks: f-chunk fc lives at cols [fc*D, (fc+1)*D)
            wq = persist.tile([128, FC * D], bf16)
            wv = persist.tile([128, FC * D], bf16)
            for wsrc, dst in ((wqkT, wq), (wvT, wv)):
                for fc in range(FC):
                    nc.gpsimd.dma_start(
                        out=dst[:, fc * D:(fc + 1) * D],
                        in_=wsrc[fc * 128:(fc + 1) * 128, :])

            # b_qk as [p, dchunk]: bqk_t[p, dc] = b_qk[dc*128 + p]
            bqk_t = persist.tile([128, DC], f32)
            nc.gpsimd.dma_start(out=bqk_t, in_=bqk.rearrange("(c p) -> p c", p=128))

            # b_v broadcast across partitions: [128, D]
            bv_bc = persist.tile([128, D], f32)
            bv_ap = bv[:]
            bv_bcast = bass.AP(
                tensor=bv_ap.tensor, offset=bv_ap.offset,
                ap=[[0, 128], *bv_ap.ap],
            )
            nc.gpsimd.dma_start(out=bv_bc, in_=bv_bcast)

            ones_f = persist.tile([128, 1], f32)
            nc.vector.memset(ones_f, 1.0)

            # ---- persistent big tensors ----
            # K^T: d-chunk dc at cols [dc*N, (dc+1)*N)   (bf16, 64KB/partition)
            kt = persist.tile([128, DC * N], bf16)
            # V: c-chunk cc at cols [cc*D, (cc+1)*D)     (bf16, 64KB/partition)
            v = persist.tile([128, CC * D], bf16)
            # local Q^T: d-chunk dc at cols [dc*NL, (dc+1)*NL)
            qtl = persist.tile([128, DC * NL], bf16)

            # ---- phase 1a: local Q^T (small, warms up PE) ----
            etl = work.tile([128, FC * NL], bf16, tag="etl", bufs=1)
            for fc in range(FC):
                nc.gpsimd.dma_start(
                    out=etl[:, fc * NL:(fc + 1) * NL],
                    in_=embTl[fc * 128:(fc + 1) * 128, :])
            for dc in range(DC):
                for nb in range(NL // 512):
                    qtl_ps = ps.tile([128, 512], f32, tag="mm_ps")
                    for fc in range(FC):
                        nc.tensor.matmul(
                            qtl_ps,
                            wq[:, fc * D + dc * 128: fc * D + (dc + 1) * 128],
                            etl[:, fc * NL + nb * 512: fc * NL + (nb + 1) * 512],
                            start=(fc == 0), stop=(fc == FC - 1),
                        )
                    nc.scalar.activation(
                        out=qtl[:, dc * NL + nb * 512: dc * NL + (nb + 1) * 512],
                        in_=qtl_ps,
                        func=mybir.ActivationFunctionType.Identity,
                        bias=bqk_t[:, dc:dc + 1],
                    )

            # ---- phase 1b: full K^T and V, in n-octants ----
            for oct_ in range(NOCT):
                n0 = oct_ * OW
                # load + cast one octant of embT: f-chunk fc at cols [fc*OW, ...)
                et = work.tile([128, FC * OW], bf16, tag="et")
                for fc in range(FC):
                    nc.gpsimd.dma_start(
                        out=et[:, fc * OW:(fc + 1) * OW],
                        in_=embT[fc * 128:(fc + 1) * 128, n0:n0 + OW],
                    )

                # K^T octant: for each d-chunk, 2 n-blocks of 512
                for dc in range(DC):
                    for nb in range(OW // 512):
                        kt_ps = ps.tile([128, 512], f32, tag="mm_ps")
                        for fc in range(FC):
                            nc.tensor.matmul(
                                kt_ps,
                                wq[:, fc * D + dc * 128: fc * D + (dc + 1) * 128],
                                et[:, fc * OW + nb * 512: fc * OW + (nb + 1) * 512],
                                start=(fc == 0), stop=(fc == FC - 1),
                            )
                        col = dc * N + n0 + nb * 512
                        nc.scalar.activation(
                            out=kt[:, col:col + 512],
                            in_=kt_ps,
                            func=mybir.ActivationFunctionType.Identity,
                            bias=bqk_t[:, dc:dc + 1],
                        )

                # V octant: 8 c-chunks of 128
                for ci in range(OW // 128):
                    v_ps = ps.tile([128, 512], f32, tag="mm_ps")
                    for fc in range(FC):
                        nc.tensor.matmul(
                            v_ps,
                            et[:, fc * OW + ci * 128: fc * OW + (ci + 1) * 128],
                            wv[:, fc * D:(fc + 1) * D],
                            start=(fc == 0), stop=(fc == FC - 1),
                        )
                    cc = oct_ * (OW // 128) + ci
                    nc.scalar.activation(
                        out=v[:, cc * D:(cc + 1) * D],
                        in_=v_ps,
                        func=mybir.ActivationFunctionType.Copy,
                    )

            # ---- phase 2: attention, 2 row-blocks of 512 ----
            for rb in range(RB):
                r0 = rb * 512
                pv_ps = [
                    ps.tile([128, D], f32, tag="pv_ps", bufs=4, name=f"pv{rb}_{j}")
                    for j in range(4)
                ]
                # l accumulated on the vector engine (PE stays on the big
                # matmuls); a final tiny matmul per r-sub reduces partitions.
                lacc = work.tile([128, 512], f32, tag="lacc", bufs=2)
                for cc in range(CC):
                    st_ps = ps.tile([128, 512], f32, tag="mm_ps")
                    for dc in range(DC):
                        nc.tensor.matmul(
                            st_ps,
                            kt[:, dc * N + cc * 128: dc * N + (cc + 1) * 128],
                            qtl[:, dc * NL + r0: dc * NL + r0 + 512],
                            start=(dc == 0), stop=(dc == DC - 1),
                        )
                    p_t = work.tile([128, 512], bf16, tag="p_t", bufs=3)
                    nc.scalar.activation(
                        out=p_t, in_=st_ps,
                        func=mybir.ActivationFunctionType.Exp,
                        scale=SCALE,
                    )
                    for j in range(4):
                        nc.tensor.matmul(
                            pv_ps[j],
                            p_t[:, j * 128:(j + 1) * 128],
                            v[:, cc * D:(cc + 1) * D],
                            start=(cc == 0), stop=(cc == CC - 1),
                        )
                    if cc == 0:
                        nc.vector.tensor_copy(out=lacc, in_=p_t)
                    else:
                        nc.vector.tensor_add(lacc, lacc, p_t)

                l_ps = ps.tile([128, 4], f32, tag="l_ps", bufs=1)
                for j in range(4):
                    nc.tensor.matmul(
                        l_ps[:, j:j + 1],
                        lacc[:, j * 128:(j + 1) * 128],
                        ones_f,
                        start=True, stop=True, skip_group_check=True,
                    )
                linv = work.tile([128, 4], f32, tag="linv")
                nc.vector.reciprocal(out=linv, in_=l_ps)
                for j in range(4):
                    o_t = work.tile([128, D], f32, tag="o_t", bufs=3)
                    nc.scalar.activation(
                        out=o_t, in_=pv_ps[j],
                        func=mybir.ActivationFunctionType.Copy,
                        scale=linv[:, j:j + 1],
                    )
                    nc.vector.tensor_add(o_t, o_t, bv_bc)
                    nc.gpsimd.dma_start(
                        out=out[r0 + j * 128: r0 + (j + 1) * 128, :], in_=o_t)

    nc.compile()
    return nc


def _get_nc():
    global _NC
    if _NC is None:
        _NC = build_kernel()
    return _NC


def kernel(embedding, W_qk, b_qk, W_v, b_v):
    global LAST_RESULT
    embedding = np.ascontiguousarray(np.asarray(embedding, dtype=np.float32))
    embT = np.ascontiguousarray(embedding.T).astype(ml_dtypes.bfloat16)
    wqkT = np.ascontiguousarray(np.asarray(W_qk, dtype=np.float32).T).astype(ml_dtypes.bfloat16)
    wvT = np.ascontiguousarray(np.asarray(W_v, dtype=np.float32).T).astype(ml_dtypes.bfloat16)
    bqk = np.ascontiguousarray(np.asarray(b_qk, dtype=np.float32))
    bv = np.ascontiguousarray(np.asarray(b_v, dtype=np.float32))

    in_maps = [
        {
            "embT": embT,
            "embTl": np.ascontiguousarray(embT[:, i * NL:(i + 1) * NL]),
            "wqkT": wqkT,
            "wvT": wvT,
            "bqk": bqk,
            "bv": bv,
        }
        for i in range(CORES)
    ]

    nc = _get_nc()
    res = run_bass_kernel_spmd(nc, in_maps, core_ids=list(range(CORES)))
    LAST_RESULT = res
    return np.concatenate(
        [np.asarray(res.results[i]["out"]) for i in range(CORES)], axis=0
    )


# revision 13
# speedup vs baseline: 1.0748x; 1.0120x over previous
"""Trainium2 8-core attention kernel (N=8192, D=512, Q==K shared projection).

Strategy: sequence-parallel attention. Each core receives the FULL transposed
embedding (replicated) plus its local 1024-column slice, computes full
K^T = (E @ W_qk^T + b_qk)^T and V = E @ W_v^T locally in bf16 (projections are
~6% of total FLOPs so replicating them beats a collective), then computes its
own [1024, 8192] score tile in the TRANSPOSED layout S^T[c, r].  In that
layout the exp'd scores are directly the lhsT of the P@V matmul (no on-device
transposes at all), and the softmax denominator comes from a ones-vector
matmul.  Softmax max-subtraction is skipped: scores are bounded (~13) so exp
stays well inside fp32/bf16 range; V's bias is folded in after normalization
(P@(V+b) = P@V + l*b  =>  out = P@V/l + b_v).
"""

import ml_dtypes
import numpy as np

import concourse.bass as bass
import concourse.mybir as mybir
import concourse.tile as tile
from concourse import bacc
from concourse.bass_utils import run_bass_kernel_spmd

N = 8192          # sequence length
F = 512           # input features
D = 512           # output features (head dim)
CORES = 8
NL = N // CORES   # local rows per core (1024)
SCALE = 1.0 / float(np.sqrt(D))

FC = F // 128     # 4 f-chunks (contraction for projections)
DC = D // 128     # 4 d-chunks (partition chunks of K^T)
NOCT = 16         # process n in 16 slabs during projections
OW = N // NOCT    # 512 slab width
RB = NL // 512    # 2 row-blocks of 512
CC = N // 128     # 64 column chunks in attention

f32 = mybir.dt.float32
bf16 = mybir.dt.bfloat16

_NC = None
LAST_RESULT = None


def build_kernel():
    nc = bacc.Bacc(target_bir_lowering=False)

    embT = nc.declare_dram_parameter("embT", [F, N], bf16, isOutput=False)
    embTl = nc.declare_dram_parameter("embTl", [F, NL], bf16, isOutput=False)
    wqkT = nc.declare_dram_parameter("wqkT", [F, D], bf16, isOutput=False)
    wvT = nc.declare_dram_parameter("wvT", [F, D], bf16, isOutput=False)
    bqk = nc.declare_dram_parameter("bqk", [D], f32, isOutput=False)
    bv = nc.declare_dram_parameter("bv", [D], f32, isOutput=False)
    out = nc.declare_dram_parameter("out", [NL, D], f32, isOutput=True)

    with tile.TileContext(nc) as tc:
        with (
            tc.tile_pool(name="persist", bufs=1) as persist,
            tc.tile_pool(name="work", bufs=2) as work,
            tc.tile_pool(name="ps", bufs=2, space="PSUM") as ps,
        ):
            # ---- constants / weights ----
            # w*T chunks: f-chunk fc lives at cols [fc*D, (fc+1)*D)
            wq = persist.tile([128, FC * D], bf16)
            wv = persist.tile([128, FC * D], bf16)
            for wsrc, dst in ((wqkT, wq), (wvT, wv)):
                for fc in range(FC):
                    nc.sync.dma_start(
                        out=dst[:, fc * D:(fc + 1) * D],
                        in_=wsrc[fc * 128:(fc + 1) * 128, :])

            # b_qk as [p, dchunk]: bqk_t[p, dc] = b_qk[dc*128 + p]
            bqk_t = persist.tile([128, DC], f32)
            nc.gpsimd.dma_start(out=bqk_t, in_=bqk.rearrange("(c p) -> p c", p=128))

            # b_v broadcast across partitions: [128, D]
            bv_bc = persist.tile([128, D], f32)
            bv_ap = bv[:]
            bv_bcast = bass.AP(
                tensor=bv_ap.tensor, offset=bv_ap.offset,
                ap=[[0, 128], *bv_ap.ap],
            )
            nc.gpsimd.dma_start(out=bv_bc, in_=bv_bcast)

            ones_f = persist.tile([128, 1], f32)
            nc.vector.memset(ones_f, 1.0)

            # ---- persistent big tensors ----
            # K^T: d-chunk dc at cols [dc*N, (dc+1)*N)   (bf16, 64KB/partition)
            kt = persist.tile([128, DC * N], bf16)
            # V: c-chunk cc at cols [cc*D, (cc+1)*D)     (bf16, 64KB/partition)
            v = persist.tile([128, CC * D], bf16)
            # local Q^T: d-chunk dc at cols [dc*NL, (dc+1)*NL)
            qtl = persist.tile([128, DC * NL], bf16)

            # ---- phase 1a: local Q^T (small, warms up PE) ----
            etl = work.tile([128, FC * NL], bf16, tag="etl", bufs=1)
            for fc in range(FC):
                nc.sync.dma_start(
                    out=etl[:, fc * NL:(fc + 1) * NL],
                    in_=embTl[fc * 128:(fc + 1) * 128, :])
            for dc in range(DC):
                for nb in range(NL // 512):
                    qtl_ps = ps.tile([128, 512], f32, tag="mm_ps")
                    for fc in range(FC):
                        nc.tensor.matmul(
                            qtl_ps,
                            wq[:, fc * D + dc * 128: fc * D + (dc + 1) * 128],
                            etl[:, fc * NL + nb * 512: fc * NL + (nb + 1) * 512],
                            start=(fc == 0), stop=(fc == FC - 1),
                        )
                    nc.scalar.activation(
                        out=qtl[:, dc * NL + nb * 512: dc * NL + (nb + 1) * 512],
                        in_=qtl_ps,
                        func=mybir.ActivationFunctionType.Identity,
                        bias=bqk_t[:, dc:dc + 1],
                    )

            # ---- phase 1b: full K^T and V, in n-octants ----
            for oct_ in range(NOCT):
                n0 = oct_ * OW
                # load + cast one octant of embT: f-chunk fc at cols [fc*OW, ...)
                et = work.tile([128, FC * OW], bf16, tag="et")
                for fc in range(FC):
                    nc.sync.dma_start(
                        out=et[:, fc * OW:(fc + 1) * OW],
                        in_=embT[fc * 128:(fc + 1) * 128, n0:n0 + OW],
                    )

                # K^T octant: for each d-chunk, 2 n-blocks of 512
                for dc in range(DC):
                    for nb in range(OW // 512):
                        kt_ps = ps.tile([128, 512], f32, tag="mm_ps")
                        for fc in range(FC):
                            nc.tensor.matmul(
                                kt_ps,
                                wq[:, fc * D + dc * 128: fc * D + (dc + 1) * 128],
                                et[:, fc * OW + nb * 512: fc * OW + (nb + 1) * 512],
                                start=(fc == 0), stop=(fc == FC - 1),
                            )
                        col = dc * N + n0 + nb * 512
                        nc.scalar.activation(
                            out=kt[:, col:col + 512],
                            in_=kt_ps,
                            func=mybir.ActivationFunctionType.Identity,
                            bias=bqk_t[:, dc:dc + 1],
                        )

                # V octant: 8 c-chunks of 128
                for ci in range(OW // 128):
                    v_ps = ps.tile([128, 512], f32, tag="mm_ps")
                    for fc in range(FC):
                        nc.tensor.matmul(
                            v_ps,
                            et[:, fc * OW + ci * 128: fc * OW + (ci + 1) * 128],
                            wv[:, fc * D:(fc + 1) * D],
                            start=(fc == 0), stop=(fc == FC - 1),
                        )
                    cc = oct_ * (OW // 128) + ci
                    nc.scalar.activation(
                        out=v[:, cc * D:(cc + 1) * D],
                        in_=v_ps,
                        func=mybir.ActivationFunctionType.Copy,
                    )

            # ---- phase 2: attention, 2 row-blocks of 512 ----
            for rb in range(RB):
                r0 = rb * 512
                pv_ps = [
                    ps.tile([128, D], f32, tag="pv_ps", bufs=4, name=f"pv{rb}_{j}")
                    for j in range(4)
                ]
                # l accumulated on the vector engine (PE stays on the big
                # matmuls); a final tiny matmul per r-sub reduces partitions.
                lacc = work.tile([128, 512], f32, tag="lacc", bufs=2)
                for cc in range(CC):
                    st_ps = ps.tile([128, 512], f32, tag="mm_ps")
                    for dc in range(DC):
                        nc.tensor.matmul(
                            st_ps,
                            kt[:, dc * N + cc * 128: dc * N + (cc + 1) * 128],
                            qtl[:, dc * NL + r0: dc * NL + r0 + 512],
                            start=(dc == 0), stop=(dc == DC - 1),
                        )
                    p_t = work.tile([128, 512], bf16, tag="p_t", bufs=3)
                    nc.scalar.activation(
                        out=p_t, in_=st_ps,
                        func=mybir.ActivationFunctionType.Exp,
                        scale=SCALE,
                    )
                    for j in range(4):
                        nc.tensor.matmul(
                            pv_ps[j],
                            p_t[:, j * 128:(j + 1) * 128],
                            v[:, cc * D:(cc + 1) * D],
                            start=(cc == 0), stop=(cc == CC - 1),
                        )
                    if cc == 0:
                        nc.vector.tensor_copy(out=lacc, in_=p_t)
                    else:
                        nc.vector.tensor_add(lacc, lacc, p_t)

                l_ps = ps.tile([128, 4], f32, tag="l_ps", bufs=1)
                for j in range(4):
                    nc.tensor.matmul(
                        l_ps[:, j:j + 1],
                        lacc[:, j * 128:(j + 1) * 128],
                        ones_f,
                        start=True, stop=True, skip_group_check=True,
                    )
                linv = work.tile([128, 4], f32, tag="linv")
                nc.vector.reciprocal(out=linv, in_=l_ps)
                for j in range(4):
                    o_t = work.tile([128, D], f32, tag="o_t", bufs=3)
                    nc.scalar.activation(
                        out=o_t, in_=pv_ps[j],
                        func=mybir.ActivationFunctionType.Copy,
                        scale=linv[:, j:j + 1],
                    )
                    nc.vector.tensor_add(o_t, o_t, bv_bc)
                    nc.sync.dma_start(
                        out=out[r0 + j * 128: r0 + (j + 1) * 128, :], in_=o_t)

    nc.compile()
    return nc


def _get_nc():
    global _NC
    if _NC is None:
        _NC = build_kernel()
    return _NC


def kernel(embedding, W_qk, b_qk, W_v, b_v):
    global LAST_RESULT
    embedding = np.ascontiguousarray(np.asarray(embedding, dtype=np.float32))
    embT = np.ascontiguousarray(embedding.T).astype(ml_dtypes.bfloat16)
    wqkT = np.ascontiguousarray(np.asarray(W_qk, dtype=np.float32).T).astype(ml_dtypes.bfloat16)
    wvT = np.ascontiguousarray(np.asarray(W_v, dtype=np.float32).T).astype(ml_dtypes.bfloat16)
    bqk = np.ascontiguousarray(np.asarray(b_qk, dtype=np.float32))
    bv = np.ascontiguousarray(np.asarray(b_v, dtype=np.float32))

    in_maps = [
        {
            "embT": embT,
            "embTl": np.ascontiguousarray(embT[:, i * NL:(i + 1) * NL]),
            "wqkT": wqkT,
            "wvT": wvT,
            "bqk": bqk,
            "bv": bv,
        }
        for i in range(CORES)
    ]

    nc = _get_nc()
    res = run_bass_kernel_spmd(nc, in_maps, core_ids=list(range(CORES)))
    LAST_RESULT = res
    return np.concatenate(
        [np.asarray(res.results[i]["out"]) for i in range(CORES)], axis=0
    )


# revision 14
# speedup vs baseline: 1.4849x; 1.3816x over previous
"""Trainium2 8-core attention kernel (N=8192, D=512, Q==K shared projection).

Strategy: sequence-parallel attention. Each core receives the FULL transposed
embedding (replicated) plus its local 1024-column slice, computes full
K^T = (E @ W_qk^T + b_qk)^T and V = E @ W_v^T locally in bf16 (projections are
~6% of total FLOPs so replicating them beats a collective), then computes its
own [1024, 8192] score tile in the TRANSPOSED layout S^T[c, r].  In that
layout the exp'd scores are directly the lhsT of the P@V matmul (no on-device
transposes at all), and the softmax denominator comes from a ones-vector
matmul.  Softmax max-subtraction is skipped: scores are bounded (~13) so exp
stays well inside fp32/bf16 range; V's bias is folded in after normalization
(P@(V+b) = P@V + l*b  =>  out = P@V/l + b_v).
"""

import ml_dtypes
import numpy as np

import concourse.bass as bass
import concourse.mybir as mybir
import concourse.tile as tile
from concourse import bacc
from concourse.bass_utils import run_bass_kernel_spmd

N = 8192          # sequence length
F = 512           # input features
D = 512           # output features (head dim)
CORES = 8
NL = N // CORES   # local rows per core (1024)
SCALE = 1.0 / float(np.sqrt(D))

FC = F // 128     # 4 f-chunks (contraction for projections)
DC = D // 128     # 4 d-chunks (partition chunks of K^T)
NOCT = 16         # process n in 16 slabs during projections
OW = N // NOCT    # 512 slab width
RB = NL // 512    # 2 row-blocks of 512
CC = N // 128     # 64 column chunks in attention

f32 = mybir.dt.float32
bf16 = mybir.dt.bfloat16

_NC = None
LAST_RESULT = None


def build_kernel():
    nc = bacc.Bacc(target_bir_lowering=False)

    embT = nc.declare_dram_parameter("embT", [F, N], bf16, isOutput=False)
    embTl = nc.declare_dram_parameter("embTl", [F, NL], bf16, isOutput=False)
    wqkT = nc.declare_dram_parameter("wqkT", [F, D], bf16, isOutput=False)
    wvT = nc.declare_dram_parameter("wvT", [F, D], bf16, isOutput=False)
    bqk = nc.declare_dram_parameter("bqk", [D], f32, isOutput=False)
    bv = nc.declare_dram_parameter("bv", [D], f32, isOutput=False)
    out = nc.declare_dram_parameter("out", [NL, D], f32, isOutput=True)

    with tile.TileContext(nc) as tc:
        with (
            tc.tile_pool(name="persist", bufs=1) as persist,
            tc.tile_pool(name="work", bufs=2) as work,
            tc.tile_pool(name="ps", bufs=2, space="PSUM") as ps,
        ):
            # ---- constants / weights ----
            # w*T chunks: f-chunk fc lives at cols [fc*D, (fc+1)*D)
            wq = persist.tile([128, FC * D], bf16)
            wv = persist.tile([128, FC * D], bf16)
            for wsrc, dst in ((wqkT, wq), (wvT, wv)):
                for fc in range(FC):
                    nc.sync.dma_start(
                        out=dst[:, fc * D:(fc + 1) * D],
                        in_=wsrc[fc * 128:(fc + 1) * 128, :])

            # b_qk as [p, dchunk]: bqk_t[p, dc] = b_qk[dc*128 + p]
            bqk_t = persist.tile([128, DC], f32)
            nc.gpsimd.dma_start(out=bqk_t, in_=bqk.rearrange("(c p) -> p c", p=128))

            # b_v broadcast across partitions: [128, D]
            bv_bc = persist.tile([128, D], f32)
            bv_ap = bv[:]
            bv_bcast = bass.AP(
                tensor=bv_ap.tensor, offset=bv_ap.offset,
                ap=[[0, 128], *bv_ap.ap],
            )
            nc.gpsimd.dma_start(out=bv_bc, in_=bv_bcast)

            ones_f = persist.tile([128, 1], f32)
            nc.vector.memset(ones_f, 1.0)

            # ---- persistent big tensors ----
            # K^T: d-chunk dc at cols [dc*N, (dc+1)*N)   (bf16, 64KB/partition)
            kt = persist.tile([128, DC * N], bf16)
            # V: c-chunk cc at cols [cc*D, (cc+1)*D)     (bf16, 64KB/partition)
            v = persist.tile([128, CC * D], bf16)
            # local Q^T: d-chunk dc at cols [dc*NL, (dc+1)*NL)
            qtl = persist.tile([128, DC * NL], bf16)

            # ---- phase 1a: local Q^T (small, warms up PE) ----
            etl = work.tile([128, FC * NL], bf16, tag="etl", bufs=1)
            for fc in range(FC):
                nc.sync.dma_start(
                    out=etl[:, fc * NL:(fc + 1) * NL],
                    in_=embTl[fc * 128:(fc + 1) * 128, :])
            for dc in range(DC):
                for nb in range(NL // 512):
                    qtl_ps = ps.tile([128, 512], f32, tag="mm_ps")
                    for fc in range(FC):
                        nc.tensor.matmul(
                            qtl_ps,
                            wq[:, fc * D + dc * 128: fc * D + (dc + 1) * 128],
                            etl[:, fc * NL + nb * 512: fc * NL + (nb + 1) * 512],
                            start=(fc == 0), stop=(fc == FC - 1),
                        )
                    nc.scalar.activation(
                        out=qtl[:, dc * NL + nb * 512: dc * NL + (nb + 1) * 512],
                        in_=qtl_ps,
                        func=mybir.ActivationFunctionType.Identity,
                        bias=bqk_t[:, dc:dc + 1],
                    )

            # ---- phase 1b: full K^T and V, in n-octants ----
            for oct_ in range(NOCT):
                n0 = oct_ * OW
                # load + cast one octant of embT: f-chunk fc at cols [fc*OW, ...)
                et = work.tile([128, FC * OW], bf16, tag="et", bufs=4)
                for fc in range(FC):
                    nc.sync.dma_start(
                        out=et[:, fc * OW:(fc + 1) * OW],
                        in_=embT[fc * 128:(fc + 1) * 128, n0:n0 + OW],
                    )

                # K^T octant: for each d-chunk, 2 n-blocks of 512
                for dc in range(DC):
                    for nb in range(OW // 512):
                        kt_ps = ps.tile([128, 512], f32, tag="mm_ps")
                        for fc in range(FC):
                            nc.tensor.matmul(
                                kt_ps,
                                wq[:, fc * D + dc * 128: fc * D + (dc + 1) * 128],
                                et[:, fc * OW + nb * 512: fc * OW + (nb + 1) * 512],
                                start=(fc == 0), stop=(fc == FC - 1),
                            )
                        col = dc * N + n0 + nb * 512
                        nc.scalar.activation(
                            out=kt[:, col:col + 512],
                            in_=kt_ps,
                            func=mybir.ActivationFunctionType.Identity,
                            bias=bqk_t[:, dc:dc + 1],
                        )

                # V octant: 8 c-chunks of 128
                for ci in range(OW // 128):
                    v_ps = ps.tile([128, 512], f32, tag="mm_ps")
                    for fc in range(FC):
                        nc.tensor.matmul(
                            v_ps,
                            et[:, fc * OW + ci * 128: fc * OW + (ci + 1) * 128],
                            wv[:, fc * D:(fc + 1) * D],
                            start=(fc == 0), stop=(fc == FC - 1),
                        )
                    cc = oct_ * (OW // 128) + ci
                    nc.scalar.activation(
                        out=v[:, cc * D:(cc + 1) * D],
                        in_=v_ps,
                        func=mybir.ActivationFunctionType.Copy,
                    )

            # ---- phase 2: attention, 2 row-blocks of 512 ----
            for rb in range(RB):
                r0 = rb * 512
                pv_ps = [
                    ps.tile([128, D], f32, tag="pv_ps", bufs=4, name=f"pv{rb}_{j}")
                    for j in range(4)
                ]
                # l accumulated on the vector engine (PE stays on the big
                # matmuls); a final tiny matmul per r-sub reduces partitions.
                lacc = work.tile([128, 512], f32, tag="lacc", bufs=2)
                for cc in range(CC):
                    st_ps = ps.tile([128, 512], f32, tag="mm_ps")
                    for dc in range(DC):
                        nc.tensor.matmul(
                            st_ps,
                            kt[:, dc * N + cc * 128: dc * N + (cc + 1) * 128],
                            qtl[:, dc * NL + r0: dc * NL + r0 + 512],
                            start=(dc == 0), stop=(dc == DC - 1),
                        )
                    p_t = work.tile([128, 512], bf16, tag="p_t", bufs=3)
                    nc.scalar.activation(
                        out=p_t, in_=st_ps,
                        func=mybir.ActivationFunctionType.Exp,
                        scale=SCALE,
                    )
                    for j in range(4):
                        nc.tensor.matmul(
                            pv_ps[j],
                            p_t[:, j * 128:(j + 1) * 128],
                            v[:, cc * D:(cc + 1) * D],
                            start=(cc == 0), stop=(cc == CC - 1),
                        )
                    if cc == 0:
                        nc.vector.tensor_copy(out=lacc, in_=p_t)
                    else:
                        nc.vector.tensor_add(lacc, lacc, p_t)

                l_ps = ps.tile([128, 4], f32, tag="l_ps", bufs=1)
                for j in range(4):
                    nc.tensor.matmul(
                        l_ps[:, j:j + 1],
                        lacc[:, j * 128:(j + 1) * 128],
                        ones_f,
                        start=True, stop=True, skip_group_check=True,
                    )
                linv = work.tile([128, 4], f32, tag="linv")
                nc.vector.reciprocal(out=linv, in_=l_ps)
                for j in range(4):
                    o_t = work.tile([128, D], f32, tag="o_t", bufs=3)
                    nc.vector.scalar_tensor_tensor(
                        out=o_t, in0=pv_ps[j], scalar=linv[:, j:j + 1],
                        in1=bv_bc, op0=mybir.AluOpType.mult,
                        op1=mybir.AluOpType.add,
                    )
                    nc.sync.dma_start(
                        out=out[r0 + j * 128: r0 + (j + 1) * 128, :], in_=o_t)

    nc.compile()
    return nc


def _get_nc():
    global _NC
    if _NC is None:
        _NC = build_kernel()
    return _NC


def kernel(embedding, W_qk, b_qk, W_v, b_v):
    global LAST_RESULT
    embedding = np.ascontiguousarray(np.asarray(embedding, dtype=np.float32))
    embT = np.ascontiguousarray(embedding.T).astype(ml_dtypes.bfloat16)
    wqkT = np.ascontiguousarray(np.asarray(W_qk, dtype=np.float32).T).astype(ml_dtypes.bfloat16)
    wvT = np.ascontiguousarray(np.asarray(W_v, dtype=np.float32).T).astype(ml_dtypes.bfloat16)
    bqk = np.ascontiguousarray(np.asarray(b_qk, dtype=np.float32))
    bv = np.ascontiguousarray(np.asarray(b_v, dtype=np.float32))

    in_maps = [
        {
            "embT": embT,
            "embTl": np.ascontiguousarray(embT[:, i * NL:(i + 1) * NL]),
            "wqkT": wqkT,
            "wvT": wvT,
            "bqk": bqk,
            "bv": bv,
        }
        for i in range(CORES)
    ]

    nc = _get_nc()
    res = run_bass_kernel_spmd(nc, in_maps, core_ids=list(range(CORES)))
    LAST_RESULT = res
    return np.concatenate(
        [np.asarray(res.results[i]["out"]) for i in range(CORES)], axis=0
    )


# revision 15
# speedup vs baseline: 1.5028x; 1.0121x over previous
"""Trainium2 8-core attention kernel v6 (N=8192, D=512, Q==K shared projection).

Projection-free formulation.  Because softmax is invariant to per-row
constants,

    scores = Q K^T = E G E^T + alpha_c + (row-constants that cancel),
    G = W_qk^T W_qk,  alpha = E (W_qk^T b_qk)

so the scores contract the RAW embedding transpose against a G-transformed
local slice (no K^T projection), with alpha folded into the exp bias.  On the
value side,

    attn @ V = (attn @ E) W_v^T + b_v

so P@E is accumulated (transposed: lhsT=E_chunk, rhs=P_chunk — no on-device
transposes) and the tiny [1024,512]x[512,512] W_v projection happens once per
core AFTER the softmax.  Both big projections (2 x 131K PE cycles) vanish.

Per-core layout: sequence-parallel — each core owns 1024 output rows,
the embedding (both layouts) is replicated, bf16 operands / f32 accumulate.
"""

import ml_dtypes
import numpy as np

import concourse.bass as bass
import concourse.mybir as mybir
import concourse.tile as tile
from concourse import bacc
from concourse.bass_utils import run_bass_kernel_spmd

N = 8192          # sequence length
F = 512           # input features
D = 512           # output features (head dim)
CORES = 8
NL = N // CORES   # local rows per core (1024)
SCALE = 1.0 / float(np.sqrt(D))

FC = F // 128     # 4 f-chunks
DC = D // 128     # 4 d-chunks
RB = NL // 512    # 2 row-blocks of 512
CC = N // 128     # 64 column chunks

f32 = mybir.dt.float32
bf16 = mybir.dt.bfloat16

_NC = None
LAST_RESULT = None


def build_kernel():
    nc = bacc.Bacc(target_bir_lowering=False)

    embT = nc.declare_dram_parameter("embT", [F, N], bf16, isOutput=False)
    emb = nc.declare_dram_parameter("emb", [N, F], bf16, isOutput=False)
    embTl = nc.declare_dram_parameter("embTl", [F, NL], bf16, isOutput=False)
    wqk = nc.declare_dram_parameter("wqk", [D, F], bf16, isOutput=False)
    wvT = nc.declare_dram_parameter("wvT", [F, D], bf16, isOutput=False)
    bqk = nc.declare_dram_parameter("bqk", [D], f32, isOutput=False)
    bv = nc.declare_dram_parameter("bv", [D], f32, isOutput=False)
    out = nc.declare_dram_parameter("out", [NL, D], f32, isOutput=True)

    with tile.TileContext(nc) as tc:
        with (
            tc.tile_pool(name="persist", bufs=1) as persist,
            tc.tile_pool(name="work", bufs=2) as work,
            tc.tile_pool(name="ps", bufs=2, space="PSUM") as ps,
        ):
            # ---- small constants ----
            wn = persist.tile([128, DC * F], bf16)     # W_qk, d-chunk dc at cols dc*F
            wv = persist.tile([128, FC * D], bf16)     # W_v^T, f-chunk fc at cols fc*D
            for fc in range(FC):
                nc.sync.dma_start(
                    out=wn[:, fc * F:(fc + 1) * F],
                    in_=wqk[fc * 128:(fc + 1) * 128, :])
                nc.sync.dma_start(
                    out=wv[:, fc * D:(fc + 1) * D],
                    in_=wvT[fc * 128:(fc + 1) * 128, :])

            bqk_d = persist.tile([128, DC], f32)
            nc.gpsimd.dma_start(out=bqk_d, in_=bqk.rearrange("(c p) -> p c", p=128))
            bqk_b = persist.tile([128, DC], bf16)
            nc.vector.tensor_copy(out=bqk_b, in_=bqk_d)

            bv_bc = persist.tile([128, D], f32)
            bv_ap = bv[:]
            nc.gpsimd.dma_start(out=bv_bc, in_=bass.AP(
                tensor=bv_ap.tensor, offset=bv_ap.offset,
                ap=[[0, 128], *bv_ap.ap]))

            ones_f = persist.tile([128, 1], f32)
            nc.vector.memset(ones_f, 1.0)

            # ---- local E^T slice ----
            etl = work.tile([128, FC * NL], bf16, tag="etl", bufs=1)
            for fc in range(FC):
                nc.sync.dma_start(
                    out=etl[:, fc * NL:(fc + 1) * NL],
                    in_=embTl[fc * 128:(fc + 1) * 128, :])

            # ---- G = W_qk^T W_qk  (bf16, f1-chunk at cols f1c*F) ----
            g_sb = persist.tile([128, FC * F], bf16)
            for f1 in range(FC):
                g_ps = ps.tile([128, F], f32, tag="mm_ps")
                for dc in range(DC):
                    nc.tensor.matmul(
                        g_ps,
                        wn[:, dc * F + f1 * 128: dc * F + (f1 + 1) * 128],
                        wn[:, dc * F:(dc + 1) * F],
                        start=(dc == 0), stop=(dc == DC - 1),
                    )
                nc.scalar.activation(
                    out=g_sb[:, f1 * F:(f1 + 1) * F], in_=g_ps,
                    func=mybir.ActivationFunctionType.Copy)

            # ---- h = SCALE * W_qk^T b_qk  (bf16 [128, FC], col fc) ----
            h_b = persist.tile([128, FC], bf16)
            for fb in range(FC):
                h_ps = ps.tile([128, 1], f32, tag="a_ps", bufs=1)
                for dc in range(DC):
                    nc.tensor.matmul(
                        h_ps,
                        wn[:, dc * F + fb * 128: dc * F + (fb + 1) * 128],
                        bqk_b[:, dc:dc + 1],
                        start=(dc == 0), stop=(dc == DC - 1),
                    )
                nc.scalar.activation(
                    out=h_b[:, fb:fb + 1], in_=h_ps,
                    func=mybir.ActivationFunctionType.Copy, scale=SCALE)

            # ---- Etilde^T_local = G^T E^T_local  (f'-chunk at cols f'c*NL) ----
            etlg = work.tile([128, FC * NL], bf16, tag="etlg", bufs=1)
            for fp in range(FC):
                for nb in range(NL // 512):
                    eg_ps = ps.tile([128, 512], f32, tag="mm_ps")
                    for fc in range(FC):
                        nc.tensor.matmul(
                            eg_ps,
                            g_sb[:, fc * F + fp * 128: fc * F + (fp + 1) * 128],
                            etl[:, fc * NL + nb * 512: fc * NL + (nb + 1) * 512],
                            start=(fc == 0), stop=(fc == FC - 1),
                        )
                    nc.scalar.activation(
                        out=etlg[:, fp * NL + nb * 512: fp * NL + (nb + 1) * 512],
                        in_=eg_ps,
                        func=mybir.ActivationFunctionType.Copy)

            # ---- big embedding tensors, streamed in column slabs ----
            # E^T: f-chunk fc at cols fc*N (64KB/part)
            et = persist.tile([128, FC * N], bf16)
            # E natural: c-chunk cc at cols cc*F (64KB/part)
            en = persist.tile([128, CC * F], bf16)
            NSLAB = 16
            SW = N // NSLAB
            for sl in range(NSLAB):
                n0 = sl * SW
                for fc in range(FC):
                    nc.sync.dma_start(
                        out=et[:, fc * N + n0: fc * N + n0 + SW],
                        in_=embT[fc * 128:(fc + 1) * 128, n0:n0 + SW])
                nc.sync.dma_start(
                    out=en[:, n0 // 128 * F: (n0 + SW) // 128 * F].rearrange(
                        "p (c f) -> p c f", f=F),
                    in_=emb[n0:n0 + SW, :].rearrange("(c p) f -> p c f", p=128))

            # alpha[:, cc] = SCALE * (E h)[cc-chunk]  (computed inside rb0 loop)
            alpha = persist.tile([128, CC], f32)

            # ---- attention: 2 row-blocks of 512 ----
            for rb in range(RB):
                r0 = rb * 512
                pvt_ps = [
                    ps.tile([128, 512], f32, tag="pvt_ps", bufs=4, name=f"pvt{rb}_{fb}")
                    for fb in range(FC)
                ]
                lacc = work.tile([128, 512], f32, tag="lacc", bufs=2)
                for cc in range(CC):
                    if rb == 0:
                        a_ps = ps.tile([128, 1], f32, tag="a_ps", bufs=1)
                        for fc in range(FC):
                            nc.tensor.matmul(
                                a_ps,
                                et[:, fc * N + cc * 128: fc * N + (cc + 1) * 128],
                                h_b[:, fc:fc + 1],
                                start=(fc == 0), stop=(fc == FC - 1),
                            )
                        nc.scalar.activation(
                            out=alpha[:, cc:cc + 1], in_=a_ps,
                            func=mybir.ActivationFunctionType.Copy)
                    st_ps = ps.tile([128, 512], f32, tag="mm_ps")
                    for fc in range(FC):
                        nc.tensor.matmul(
                            st_ps,
                            et[:, fc * N + cc * 128: fc * N + (cc + 1) * 128],
                            etlg[:, fc * NL + r0: fc * NL + r0 + 512],
                            start=(fc == 0), stop=(fc == FC - 1),
                        )
                    p_t = work.tile([128, 512], bf16, tag="p_t", bufs=3)
                    nc.scalar.activation(
                        out=p_t, in_=st_ps,
                        func=mybir.ActivationFunctionType.Exp,
                        scale=SCALE, bias=alpha[:, cc:cc + 1],
                    )
                    for fb in range(FC):
                        nc.tensor.matmul(
                            pvt_ps[fb],
                            en[:, cc * F + fb * 128: cc * F + (fb + 1) * 128],
                            p_t,
                            start=(cc == 0), stop=(cc == CC - 1),
                        )
                    if cc == 0:
                        nc.vector.tensor_copy(out=lacc, in_=p_t)
                    else:
                        nc.vector.tensor_add(lacc, lacc, p_t)

                l_ps = ps.tile([128, 4], f32, tag="l_ps", bufs=1)
                for j in range(4):
                    nc.tensor.matmul(
                        l_ps[:, j:j + 1],
                        lacc[:, j * 128:(j + 1) * 128],
                        ones_f,
                        start=True, stop=True, skip_group_check=True,
                    )
                linv = work.tile([128, 4], f32, tag="linv")
                nc.vector.reciprocal(out=linv, in_=l_ps)

                # (P@E)^T staged to SBUF, then the small W_v projection
                ptb = [
                    work.tile([128, 512], bf16, tag="ptb", bufs=8, name=f"ptb{rb}_{fb}")
                    for fb in range(FC)
                ]
                for fb in range(FC):
                    nc.scalar.activation(
                        out=ptb[fb], in_=pvt_ps[fb],
                        func=mybir.ActivationFunctionType.Copy)
                for j in range(4):
                    o_ps = ps.tile([128, D], f32, tag="mm_ps")
                    for fb in range(FC):
                        nc.tensor.matmul(
                            o_ps,
                            ptb[fb][:, j * 128:(j + 1) * 128],
                            wv[:, fb * D:(fb + 1) * D],
                            start=(fb == 0), stop=(fb == FC - 1),
                        )
                    o_t = work.tile([128, D], f32, tag="o_t", bufs=3)
                    nc.vector.scalar_tensor_tensor(
                        out=o_t, in0=o_ps, scalar=linv[:, j:j + 1],
                        in1=bv_bc, op0=mybir.AluOpType.mult,
                        op1=mybir.AluOpType.add,
                    )
                    nc.sync.dma_start(
                        out=out[r0 + j * 128: r0 + (j + 1) * 128, :], in_=o_t)

    nc.compile()
    return nc


def _get_nc():
    global _NC
    if _NC is None:
        _NC = build_kernel()
    return _NC


def kernel(embedding, W_qk, b_qk, W_v, b_v):
    global LAST_RESULT
    embedding = np.ascontiguousarray(np.asarray(embedding, dtype=np.float32))
    emb_b = embedding.astype(ml_dtypes.bfloat16)
    embT_b = np.ascontiguousarray(embedding.T).astype(ml_dtypes.bfloat16)
    wqk_b = np.ascontiguousarray(np.asarray(W_qk, dtype=np.float32)).astype(ml_dtypes.bfloat16)
    wvT_b = np.ascontiguousarray(np.asarray(W_v, dtype=np.float32).T).astype(ml_dtypes.bfloat16)
    bqk = np.ascontiguousarray(np.asarray(b_qk, dtype=np.float32))
    bv = np.ascontiguousarray(np.asarray(b_v, dtype=np.float32))

    in_maps = [
        {
            "embT": embT_b,
            "emb": emb_b,
            "embTl": np.ascontiguousarray(embT_b[:, i * NL:(i + 1) * NL]),
            "wqk": wqk_b,
            "wvT": wvT_b,
            "bqk": bqk,
            "bv": bv,
        }
        for i in range(CORES)
    ]

    nc = _get_nc()
    res = run_bass_kernel_spmd(nc, in_maps, core_ids=list(range(CORES)))
    LAST_RESULT = res
    return np.concatenate(
        [np.asarray(res.results[i]["out"]) for i in range(CORES)], axis=0
    )
